# revision 1
# baseline (speedup 1.0000x reference)
"""Multi-head self-attention (CrossAttention module with encoder_hidden_states=None)
on 8 Trainium2 NeuronCores.

Problem: hidden_states [B=4, S=2048, D=512], 8 heads x 64 dim, fp32.
    q/k/v = x @ W{q,k,v};  per-head softmax(q k^T / 8) v;  out proj + bias.

Sharding: core c handles (batch b = c//2, query-token half qh = c%2), i.e. a
1024-token query slice of one batch element. Each core computes K/V for its
full batch element (2048 tokens, duplicated across the 2 cores sharing a
batch) and the complete attention + output projection for its query slice.
Outputs are disjoint token slices -> pure concatenation, no collectives.

Everything on-chip is kept "transposed" (feature dim on partitions) so the
tensor engine never needs an explicit transpose:
    QT[d, q] = Wq^T x^T          (chunks of Wq as stationary operand)
    KT[d, k] = Wk^T x^T
    V[k, d]  = x Wv              (natural layout; chunks of x^T stationary)
    S^T[k, q] = (KT_h)^T QT_h    per head  (contraction over head dim 64)
    P^T = exp(S^T / 8)           (scalar-engine Exp, unnormalized)
    O^T[dh+1, q] = [V_h | 1]^T P^T   (ones column yields softmax denominators)
    AoT[d, q] = O^T rows scaled by 1/denominator (broadcast via gpsimd)
    out[t, d] = AoT^T Wo + bo

Matmuls run in float32r (TF32-like, 4x faster than fp32; measured norm rel
err ~2e-4 for a 512-deep contraction).
"""

import numpy as np

import concourse.bass as bass
import concourse.mybir as mybir
import concourse.tile as tile
from concourse import bacc
from concourse.bass_utils import run_bass_kernel_spmd
from contextlib import ExitStack

F32 = mybir.dt.float32
F32R = mybir.dt.float32r

B, S, D = 4, 2048, 512
H, DH = 8, 64
SCALE = DH ** -0.5  # 0.125
NCORES = 8
QS = S // 2  # query tokens per core (1024)
KC = D // 128  # contraction chunks over feature dim (4)
TC = S // 128  # token chunks of the full sequence (16)

_CACHE = {}
LAST_RESULTS = None  # BassKernelResults of the most recent kernel() call


def _build():
    nc = bacc.Bacc("TRN2", target_bir_lowering=False, debug=False,
                   enable_asserts=False)

    xk = nc.dram_tensor("xk", [D, S], F32R, kind="ExternalInput").ap()
    xq = nc.dram_tensor("xq", [D, QS], F32R, kind="ExternalInput").ap()
    wq = nc.dram_tensor("wq", [D, D], F32R, kind="ExternalInput").ap()
    wk = nc.dram_tensor("wk", [D, D], F32R, kind="ExternalInput").ap()
    wv = nc.dram_tensor("wv", [D, D], F32R, kind="ExternalInput").ap()
    wo = nc.dram_tensor("wo", [D, D], F32R, kind="ExternalInput").ap()
    bo = nc.dram_tensor("bo", [D], F32, kind="ExternalInput").ap()
    out = nc.dram_tensor("out", [QS, D], F32, kind="ExternalOutput").ap()

    with tile.TileContext(nc) as tc, ExitStack() as ctx:
        # SBUF pools (bytes/partition):
        #   xkao 4x8KB=32K  xqpt 4x4KB=16K  kt 4x8KB=32K  qt 4x4KB=16K
        #   w 16x2KB=32K    va 16x~2.2K=35K rb 2x4KB=8K   singles 2K
        #   ost 2x2KB=4K    recip ~tiny     => ~177KB of 192KB
        xkao = ctx.enter_context(tc.tile_pool(name="xkao", bufs=4))
        xqpt = ctx.enter_context(tc.tile_pool(name="xqpt", bufs=4))
        ktp = ctx.enter_context(tc.tile_pool(name="ktp", bufs=4))
        qtp = ctx.enter_context(tc.tile_pool(name="qtp", bufs=4))
        wp = ctx.enter_context(tc.tile_pool(name="wp", bufs=16))
        vap = ctx.enter_context(tc.tile_pool(name="vap", bufs=16))
        rbp = ctx.enter_context(tc.tile_pool(name="rbp", bufs=2))
        rcp = ctx.enter_context(tc.tile_pool(name="rcp", bufs=2))
        ost = ctx.enter_context(tc.tile_pool(name="ost", bufs=2))
        aop = ctx.enter_context(tc.tile_pool(name="aop", bufs=4))
        singles = ctx.enter_context(tc.tile_pool(name="singles", bufs=1))
        # PSUM: psA 3x2banks + psO 2x1bank = 8 banks
        psA = ctx.enter_context(tc.tile_pool(name="psA", bufs=3, space="PSUM"))
        psO = ctx.enter_context(tc.tile_pool(name="psO", bufs=2, space="PSUM"))

        # ---- input loads ------------------------------------------------
        xk_t, xq_t = [], []
        for kc in range(KC):
            t = xkao.tile([128, S], F32R, tag="xkao")
            nc.sync.dma_start(out=t, in_=xk[kc * 128:(kc + 1) * 128, :])
            xk_t.append(t)
            t = xqpt.tile([128, QS], F32R, tag="xqpt")
            nc.sync.dma_start(out=t, in_=xq[kc * 128:(kc + 1) * 128, :])
            xq_t.append(t)
        w_t = {}
        for name, ap in (("wq", wq), ("wk", wk), ("wv", wv), ("wo", wo)):
            w_t[name] = []
            for kc in range(KC):
                t = wp.tile([128, D], F32R, tag="wp")
                nc.sync.dma_start(out=t, in_=ap[kc * 128:(kc + 1) * 128, :])
                w_t[name].append(t)
        bo_b = singles.tile([128, D], F32)
        bo_bcast_ap = bass.AP(tensor=bo.tensor, offset=bo.offset,
                              ap=[[0, 128]] + list(bo.ap))
        nc.sync.dma_start(out=bo_b, in_=bo_bcast_ap)
        ones_h = singles.tile([128, H, 1], F32)
        nc.vector.memset(ones_h, 1.0)

        # ---- QT[d, q] = Wq^T @ x_q^T  (4 tiles [128, QS]) ---------------
        qt = []
        for dc in range(KC):
            ps = psA.tile([128, 1024], F32, tag="psA")
            for kc in range(KC):
                lhsT = w_t["wq"][kc][:, dc * 128:(dc + 1) * 128]
                for nh in range(QS // 512):
                    nc.tensor.matmul(
                        ps[:, nh * 512:(nh + 1) * 512], lhsT,
                        xq_t[kc][:, nh * 512:(nh + 1) * 512],
                        start=(kc == 0), stop=(kc == KC - 1))
            t = qtp.tile([128, QS], F32R, tag="qtp")
            nc.vector.tensor_copy(out=t, in_=ps)
            qt.append(t)

        # ---- KT[d, k] = Wk^T @ x^T  (4 tiles [128, S]) ------------------
        kt = [None] * KC

        def emit_kt(dc):
            t = ktp.tile([128, S], F32R, tag="ktp", name="kt")
            for half in range(2):
                ps = psA.tile([128, 1024], F32, tag="psA", name="ps")
                for kc in range(KC):
                    lhsT = w_t["wk"][kc][:, dc * 128:(dc + 1) * 128]
                    for nh in range(2):
                        col = half * 1024 + nh * 512
                        nc.tensor.matmul(
                            ps[:, nh * 512:(nh + 1) * 512], lhsT,
                            xk_t[kc][:, col:col + 512],
                            start=(kc == 0), stop=(kc == KC - 1))
                nc.vector.tensor_copy(
                    out=t[:, half * 1024:(half + 1) * 1024], in_=ps)
            kt[dc] = t

        emit_kt(0)

        # ---- V_aug[k, h, 0:64]=x@Wv slice, [..,64]=1  (16 tiles) --------
        va = []
        for tci in range(TC):
            ps = psO.tile([128, 512], F32, tag="psO")
            for kc in range(KC):
                nc.tensor.matmul(
                    ps, xk_t[kc][:, tci * 128:(tci + 1) * 128],
                    w_t["wv"][kc],
                    start=(kc == 0), stop=(kc == KC - 1))
            t = vap.tile([128, H, DH + 1], F32R, tag="vap")
            nc.vector.tensor_copy(
                out=t[:, :, 0:DH],
                in_=ps.rearrange("p (h d) -> p h d", h=H))
            nc.vector.tensor_copy(out=t[:, :, DH:DH + 1], in_=ones_h)
            va.append(t)

        # ---- attention; AoT[d, q] tiles [128, QS] -----------------------
        aot = []
        for hp in range(H // 2):
            aot.append(aop.tile([128, QS], F32R, tag="aop", name="aot"))
        def emit_attention(hp):
            for qb in range(2):
                q0 = qb * 512
                pso = [psO.tile([DH + 1, 512], F32, tag="psO", name="pso")
                       for _ in range(2)]
                for kcp in range(TC // 2):
                    # scores^T tile = two k-chunks x 512 queries; interleave
                    # the two heads (row groups 0/64) so the PE runs them
                    # concurrently
                    pss = [psA.tile([128, 1024], F32, tag="psA", name="pss")
                           for _ in range(2)]
                    for j in range(2):
                        tci = kcp * 2 + j
                        for hh in range(2):
                            r0 = hh * DH
                            nc.tensor.matmul(
                                pss[hh][:, j * 512:(j + 1) * 512],
                                kt[hp][r0:r0 + DH, tci * 128:(tci + 1) * 128],
                                qt[hp][r0:r0 + DH, q0:q0 + 512],
                                start=True, stop=True)
                    for hh in range(2):
                        pt = xqpt.tile([128, 1024], F32R, tag="xqpt")
                        nc.scalar.activation(
                            out=pt, in_=pss[hh],
                            func=mybir.ActivationFunctionType.Exp, scale=SCALE)
                        h = hp * 2 + hh
                        for j in range(2):
                            tci = kcp * 2 + j
                            nc.tensor.matmul(
                                pso[hh],
                                va[tci][:, h, :],
                                pt[:, j * 512:(j + 1) * 512],
                                start=(tci == 0), stop=(tci == TC - 1))
                for hh in range(2):
                    rc = rcp.tile([1, 512], F32, tag="rcp")
                    nc.vector.reciprocal(rc, pso[hh][DH:DH + 1, :])
                    rb = rbp.tile([DH, 512], F32, tag="rbp")
                    nc.gpsimd.partition_broadcast(rb, rc)
                    nc.vector.tensor_mul(
                        aot[hp][hh * DH:(hh + 1) * DH, q0:q0 + 512],
                        pso[hh][0:DH, :], rb)

        # head-pair 0 starts right after KT[0]; remaining KT chunks overlap
        # with the ACT-bound attention phase
        emit_attention(0)
        for dc in range(1, KC):
            emit_kt(dc)
        for hp in range(1, H // 2):
            emit_attention(hp)

        # ---- out[t, d] = AoT^T @ Wo + bo --------------------------------
        for tci in range(QS // 128):
            ps = psO.tile([128, 512], F32, tag="psO")
            for dc in range(KC):
                nc.tensor.matmul(
                    ps, aot[dc][:, tci * 128:(tci + 1) * 128],
                    w_t["wo"][dc],
                    start=(dc == 0), stop=(dc == KC - 1))
            ot = ost.tile([128, D], F32, tag="ost")
            nc.vector.tensor_add(ot, ps, bo_b)
            nc.sync.dma_start(out=out[tci * 128:(tci + 1) * 128, :], in_=ot)

    nc.compile()
    return nc


def kernel(hidden_states, Wq, Wk, Wv, Wo, bo):
    global LAST_RESULTS
    hidden_states = np.asarray(hidden_states, dtype=np.float32)
    Wq = np.asarray(Wq, dtype=np.float32)
    Wk = np.asarray(Wk, dtype=np.float32)
    Wv = np.asarray(Wv, dtype=np.float32)
    Wo = np.asarray(Wo, dtype=np.float32)
    bo = np.asarray(bo, dtype=np.float32)

    if "nc" not in _CACHE:
        _CACHE["nc"] = _build()
    nc = _CACHE["nc"]

    in_maps = []
    for c in range(NCORES):
        b, qh = c // 2, c % 2
        xT = np.ascontiguousarray(hidden_states[b].T)  # [D, S]
        in_maps.append({
            "xk": xT,
            "xq": np.ascontiguousarray(xT[:, qh * QS:(qh + 1) * QS]),
            "wq": Wq, "wk": Wk, "wv": Wv, "wo": Wo, "bo": bo,
        })

    res = run_bass_kernel_spmd(nc, in_maps, core_ids=list(range(NCORES)))
    LAST_RESULTS = res

    out = np.empty((B, S, D), dtype=np.float32)
    for c in range(NCORES):
        b, qh = c // 2, c % 2
        out[b, qh * QS:(qh + 1) * QS, :] = res.results[c]["out"]
    return out

